# revision 1
# baseline (speedup 1.0000x reference)
"""CLIP attention (ShareKey branch) Trainium2 Bass kernel, 8-core SPMD.

Math: in the reference, attn = softmax(scores[..., None] + share_bias, axis=-1)
where scores is constant along the softmax axis -> softmax shift-invariance
makes the q-projection / share_key / scores irrelevant. The output is exactly

    P[h]   = softmax(share_bias[h], axis=-1)            (batch independent)
    V[b]   = hidden[b] @ v_w.T + v_b
    O[b,h] = P[h] @ V[b,h]                               (per-head slice of V)
    out[b] = concat_h(O[b,h]) @ out_w.T + out_b

Sharding: data-parallel over batch (16 batches / 8 cores = 2 per core);
weights + bias replicated per core. All transposes required to feed the PE
(contraction dim on partitions) are done host-side in numpy as part of input
layout: hiddenT = hidden^T per batch, wvT = v_w.T, woT = out_w.T, biasT =
share_bias^T per head. On-device, per core:

  V[j, (b,e)]     = sum_k hiddenT[b][k, j] * wvT[k, e]         (PE fp32r)
  PT[h][j, i]     = exp(biasT[h][j, i])              (ACT, bf16 in/out, in-place)
  sumexp[h][i]    = sum_j PT[h][j, i]                          (PE, ones-mat)
  OT[b][hd, i]    = (sum_j V[j,(b,hd)] * PT[h][j,i]) / sumexp  (PE bf16 + DVE)
  out[b][i, m]    = sum_hd OT[b][hd, i] * woT[hd, m] + c[m]    (PE fp32r + DVE)
  c[m]            = v_b @ woT + out_b   (v_b folded through: P rows sum to 1)

The attention matmuls run in bf16 (fp32r matmuls require dst start_partition
0 and even moving counts, which the col-tiled per-head layout can't satisfy);
the big projections run fp32r (11-bit-mantissa fp32) at full PE rate.
"""

import numpy as np

B, S, E = 16, 577, 1024
H, D = 16, 64
NCORES = 8
BPC = B // NCORES  # batches per core

# sequence tiles (partition-dim tiles of 128, last ragged 65)
STILES = [(0, 128), (128, 256), (256, 384), (384, 512), (512, 577)]
NST = len(STILES)
ICHUNKS = [(0, 289), (289, 577)]  # moving-dim chunks of the attention matmul
NKT = E // 128  # 8 contraction tiles
NEC = E // 512  # 2 free-dim chunks of the projections


def _build_program(debug=False):
    import concourse.bass as bass
    import concourse.bacc as bacc
    import concourse.mybir as mybir
    import concourse.tile as tile

    dt = mybir.dt
    f32 = dt.float32
    f32r = dt.float32r
    bf16 = dt.bfloat16
    Exp = mybir.ActivationFunctionType.Exp
    PSUM = bass.MemorySpace.PSUM

    nc = bacc.Bacc("TRN2", target_bir_lowering=False, debug=False, num_devices=NCORES)

    hT = nc.declare_dram_parameter("hiddenT", [BPC, E, S], f32r, isOutput=False)
    wvT = nc.declare_dram_parameter("wvT", [E, E], f32r, isOutput=False)
    woT = nc.declare_dram_parameter("woT", [E, E], f32r, isOutput=False)
    vb = nc.declare_dram_parameter("v_b", [E], f32r, isOutput=False)
    ob = nc.declare_dram_parameter("out_b", [E], f32, isOutput=False)
    bT = nc.declare_dram_parameter("biasT", [H, S, S], bf16, isOutput=False)
    out = nc.declare_dram_parameter("out", [BPC, S, E], f32, isOutput=True)
    if debug:
        dbg_v = nc.declare_dram_parameter("dbg_v", [NST, 128, BPC, E], bf16, isOutput=True)
        dbg_inv = nc.declare_dram_parameter("dbg_inv", [H // 2, 128, S], f32, isOutput=True)
        dbg_ot = nc.declare_dram_parameter("dbg_ot", [BPC, NKT, 128, S], f32r, isOutput=True)

    with tile.TileContext(nc) as tc:
        with (
            tc.tile_pool(name="const", bufs=1) as const_pool,
            tc.tile_pool(name="wop", bufs=1) as wo_pool,
            tc.tile_pool(name="vsb", bufs=NST) as v_pool,
            tc.tile_pool(name="ptp", bufs=5) as pt_pool,
            tc.tile_pool(name="invp", bufs=4) as invs_pool,
            tc.tile_pool(name="i2p", bufs=2) as i2_pool,
            tc.tile_pool(name="mvp", bufs=2) as mv_pool,
            tc.tile_pool(name="osbp", bufs=2) as osb_pool,
            tc.tile_pool(name="psum", bufs=2, space=PSUM) as psum_pool,
        ):
            # ---- constants + weight loads ---------------------------------
            ones_mat = const_pool.tile([128, 64], bf16, tag="ones", name="ones")
            nc.vector.memset(ones_mat[:], 1.0)

            v_sb = [
                v_pool.tile([128, BPC, E], bf16, tag="v", name="v") for _ in STILES
            ]

            # ---- phase A: V projection (fp32r) ----------------------------
            with (
                tc.tile_pool(name="wvp", bufs=1) as wv_pool,
                tc.tile_pool(name="htp", bufs=BPC) as ht_pool,
            ):
                # interleave wv/ht[0] per k-tile so the first V-proj
                # accumulation chain starts after ~2 tiles instead of 8.7MB
                wv_t = wv_pool.tile([128, NKT, E], f32r, tag="wv", name="wv")
                ht_t = [
                    ht_pool.tile([128, NKT, S], f32r, tag="ht", name="ht")
                    for _ in range(BPC)
                ]
                for kt in range(NKT):
                    nc.sync.dma_start(wv_t[:, kt, :], wvT[bass.ts(kt, 128), :])
                    nc.sync.dma_start(ht_t[0][:, kt, :], hT[0, bass.ts(kt, 128), :])
                for b in range(1, BPC):
                    for kt in range(NKT):
                        nc.sync.dma_start(ht_t[b][:, kt, :], hT[b, bass.ts(kt, 128), :])

                # bias DMAs + in-place exp: emitted early so the DMA queue
                # streams bias under the V projection; ACT is otherwise idle
                pt_t = {}
                for h in range(H):
                    p = pt_pool.tile([128, NST, S], bf16, tag="pt", name="pt")
                    nc.sync.dma_start(
                        p[:, 0:4, :],
                        bT[h, 0:512, :].rearrange("(jt p) i -> p jt i", p=128),
                    )
                    nc.sync.dma_start(p[0:65, 4, :], bT[h, 512:577, :])
                    nc.scalar.activation(p[:, 0:4, :], p[:, 0:4, :], Exp)
                    nc.scalar.activation(p[0:65, 4, :], p[0:65, 4, :], Exp)
                    pt_t[h] = p

                for b in range(BPC):
                    for st, (s0, s1) in enumerate(STILES):
                        ssz = s1 - s0
                        for ec in range(NEC):
                            ps = psum_pool.tile([128, 512], f32, tag="ps8", name="vps", bufs=6)
                            for kt in range(NKT):
                                nc.tensor.matmul(
                                    ps[0:ssz, :],
                                    ht_t[b][:, kt, s0:s1],
                                    wv_t[:, kt, bass.ts(ec, 512)],
                                    start=(kt == 0),
                                    stop=(kt == NKT - 1),
                                )
                            nc.vector.tensor_copy(
                                v_sb[st][0:ssz, b, bass.ts(ec, 512)], ps[0:ssz, :]
                            )

                if debug:
                    for st in range(NST):
                        nc.sync.dma_start(dbg_v[st], v_sb[st][:])

            # ---- phase B: per-head-pair softmax + attention (bf16) --------
            with tc.tile_pool(name="otp", bufs=BPC * NKT) as ot_pool:
                ot_t = {}
                for b in range(BPC):
                    for kt in range(NKT):
                        ot_t[b, kt] = ot_pool.tile([128, S], f32r, tag="ot", name="ot")

                wo_t = wo_pool.tile([128, NKT, E], f32r, tag="wo", name="wo")
                for kt in range(NKT):
                    nc.sync.dma_start(wo_t[:, kt, :], woT[bass.ts(kt, 128), :])
                vb_sb = const_pool.tile([128, NKT], f32r, tag="vb", name="vb")
                nc.sync.dma_start(vb_sb[:], vb.rearrange("(a p) -> p a", p=128))
                ob_sb = const_pool.tile([1, E], f32, tag="ob", name="ob")
                nc.sync.dma_start(ob_sb[:], ob.rearrange("(a e) -> a e", a=1))

                for kt in range(NKT):
                    h0, h1 = 2 * kt, 2 * kt + 1

                    # paired sumexp: both heads concurrently via col tiling
                    inv2 = i2_pool.tile([128, S], f32, tag="i2", name="i2")
                    for (i0, i1) in ICHUNKS:
                        isz = i1 - i0
                        sps = psum_pool.tile([128, 289], f32, tag="sps", name="sps", bufs=2)
                        for jt, (j0, j1) in enumerate(STILES):
                            jsz = j1 - j0
                            nc.tensor.matmul(
                                sps[0:64, 0:isz],
                                ones_mat[0:jsz, :],
                                pt_t[h0][0:jsz, jt, i0:i1],
                                start=(jt == 0),
                                stop=(jt == NST - 1),
                                tile_position=(0, 0),
                            )
                            nc.tensor.matmul(
                                sps[64:128, 0:isz],
                                ones_mat[0:jsz, :],
                                pt_t[h1][0:jsz, jt, i0:i1],
                                start=(jt == 0),
                                stop=(jt == NST - 1),
                                tile_position=(0, 64),
                            )
                        nc.vector.reciprocal(inv2[0:1, i0:i1], sps[0:1, 0:isz])
                        nc.vector.reciprocal(inv2[64:65, i0:i1], sps[64:65, 0:isz])

                    # odd head's row lives on partition 64; hop it to
                    # partition 0 via SBUF->SBUF DMA for the broadcast
                    mv = mv_pool.tile([1, S], f32, tag="mv", name="mv")
                    nc.scalar.dma_start(mv[0:1, :], inv2[64:65, :])
                    inv_bc0 = invs_pool.tile([128, S], f32, tag="invs", name="invs")
                    nc.gpsimd.partition_broadcast(inv_bc0[:], inv2[0:1, :])
                    inv_bc1 = invs_pool.tile([128, S], f32, tag="invs", name="invs")
                    nc.gpsimd.partition_broadcast(inv_bc1[:], mv[0:1, :])

                    if debug:
                        nc.sync.dma_start(dbg_inv[kt][0:64], inv_bc0[0:64, :])
                        nc.sync.dma_start(dbg_inv[kt][64:128], inv_bc1[64:128, :])

                    # attention: psum[(h%2)*64+d, i] over j tiles, col-tiled
                    for b in range(BPC):
                        for (i0, i1) in ICHUNKS:
                            isz = i1 - i0
                            ps = psum_pool.tile([128, 289], f32, tag="ps8", name="aps", bufs=6)
                            for jt, (j0, j1) in enumerate(STILES):
                                jsz = j1 - j0
                                nc.tensor.matmul(
                                    ps[0:64, 0:isz],
                                    v_sb[jt][0:jsz, b, h0 * 64 : h0 * 64 + 64],
                                    pt_t[h0][0:jsz, jt, i0:i1],
                                    start=(jt == 0),
                                    stop=(jt == NST - 1),
                                    tile_position=(0, 0),
                                )
                                nc.tensor.matmul(
                                    ps[64:128, 0:isz],
                                    v_sb[jt][0:jsz, b, h1 * 64 : h1 * 64 + 64],
                                    pt_t[h1][0:jsz, jt, i0:i1],
                                    start=(jt == 0),
                                    stop=(jt == NST - 1),
                                    tile_position=(0, 64),
                                )
                            nc.vector.tensor_mul(
                                ot_t[b, kt][0:64, i0:i1],
                                ps[0:64, 0:isz],
                                inv_bc0[0:64, i0:i1],
                            )
                            nc.vector.tensor_mul(
                                ot_t[b, kt][64:128, i0:i1],
                                ps[64:128, 0:isz],
                                inv_bc1[64:128, i0:i1],
                            )

                if debug:
                    for b in range(BPC):
                        for kt in range(NKT):
                            nc.sync.dma_start(dbg_ot[b, kt], ot_t[b, kt][:])

                # ---- c = v_b @ woT + out_b, broadcast -----------------
                c_sb = const_pool.tile([1, E], f32, tag="c", name="c")
                c_bc = const_pool.tile([128, E], f32, tag="cbc", name="cbc")
                for mc in range(NEC):
                    cps = psum_pool.tile([128, 512], f32, tag="ps8", name="cps", bufs=6)
                    for kt in range(NKT):
                        nc.tensor.matmul(
                            cps[0:1, :],
                            vb_sb[:, kt : kt + 1],
                            wo_t[:, kt, bass.ts(mc, 512)],
                            start=(kt == 0),
                            stop=(kt == NKT - 1),
                        )
                    nc.vector.tensor_add(
                        c_sb[:, bass.ts(mc, 512)], cps[0:1, :], ob_sb[:, bass.ts(mc, 512)]
                    )
                nc.gpsimd.partition_broadcast(c_bc[:], c_sb[:])

                # ---- phase C: output projection (fp32r) -------------------
                for b in range(BPC):
                    for (s0, s1) in STILES:
                        ssz = s1 - s0
                        for mc in range(NEC):
                            ps = psum_pool.tile([128, 512], f32, tag="ps8", name="ops", bufs=6)
                            for kt in range(NKT):
                                nc.tensor.matmul(
                                    ps[0:ssz, :],
                                    ot_t[b, kt][:, s0:s1],
                                    wo_t[:, kt, bass.ts(mc, 512)],
                                    start=(kt == 0),
                                    stop=(kt == NKT - 1),
                                )
                            osb = osb_pool.tile([128, 512], f32, tag="osb", name="osb")
                            nc.vector.tensor_add(
                                osb[0:ssz, :],
                                ps[0:ssz, :],
                                c_bc[0:ssz, bass.ts(mc, 512)],
                            )
                            nc.sync.dma_start(
                                out[b, s0:s1, bass.ts(mc, 512)], osb[0:ssz, :]
                            )

    nc.finalize()
    return nc


def _to_fp32r(a):
    """Round fp32 to the fp32r format: RNE to 11 explicit mantissa bits,
    low 12 bits of the word zeroed (matches walrus fp32_to_fp32r)."""
    u = np.ascontiguousarray(a, dtype=np.float32).view(np.uint32)
    r = (u.astype(np.uint64) + 0x7FF + ((u >> 12) & 1)).astype(np.uint32) & np.uint32(
        0xFFFFF000
    )
    return r.view(np.float32)


_NC_CACHE = None


def _get_program():
    global _NC_CACHE
    if _NC_CACHE is None:
        _NC_CACHE = _build_program()
    return _NC_CACHE


def kernel(
    hidden_states,
    q_w,
    q_b,
    v_w,
    v_b,
    out_w,
    out_b,
    share_key,
    share_bias,
    layer,
    _trace=False,
):
    """Full-input / full-output entry point. q_w/q_b/share_key/layer are
    mathematically irrelevant (softmax shift invariance) and unused."""
    from concourse.bass_utils import run_bass_kernel_spmd

    hidden_states = np.ascontiguousarray(np.asarray(hidden_states, dtype=np.float32))
    v_w = np.asarray(v_w, dtype=np.float32)
    v_b = np.ascontiguousarray(np.asarray(v_b, dtype=np.float32))
    out_w = np.asarray(out_w, dtype=np.float32)
    out_b = np.ascontiguousarray(np.asarray(out_b, dtype=np.float32))
    share_bias = np.asarray(share_bias, dtype=np.float32)

    # host-side layout transforms (transposes + fp32r rounding, no math).
    hiddenT = _to_fp32r(np.ascontiguousarray(hidden_states.transpose(0, 2, 1)))
    wvT = _to_fp32r(np.ascontiguousarray(v_w.T))  # [k, e]
    woT = _to_fp32r(np.ascontiguousarray(out_w.T))  # [k, m]
    v_b = _to_fp32r(v_b)
    import ml_dtypes

    biasT = np.ascontiguousarray(
        share_bias.transpose(0, 2, 1).astype(ml_dtypes.bfloat16)
    )  # [H, j, i] bf16

    nc = _get_program()
    in_maps = []
    for c in range(NCORES):
        in_maps.append(
            {
                "hiddenT": hiddenT[c * BPC : (c + 1) * BPC],
                "wvT": wvT,
                "woT": woT,
                "v_b": v_b,
                "out_b": out_b,
                "biasT": biasT,
            }
        )
    res = run_bass_kernel_spmd(nc, in_maps, list(range(NCORES)), trace=_trace)
    out = np.concatenate([res.results[c]["out"] for c in range(NCORES)], axis=0)
    if _trace:
        kernel.last_results = res
    return out



# revision 6
# speedup vs baseline: 1.1426x; 1.1426x over previous
"""CLIP attention (ShareKey branch) Trainium2 Bass kernel, 8-core SPMD.

Math: in the reference, attn = softmax(scores[..., None] + share_bias, axis=-1)
where scores is constant along the softmax axis -> softmax shift-invariance
makes the q-projection / share_key / scores irrelevant. The output is exactly

    P[h]   = softmax(share_bias[h], axis=-1)            (batch independent)
    V[b]   = hidden[b] @ v_w.T                          (v_b folded into c)
    O[b,h] = P[h] @ V[b,h]                              (per-head slice of V)
    out[b] = concat_h(O[b,h]) @ out_w.T + c,  c = v_b @ out_w.T + out_b
    (P rows sum to 1, so the v_b contribution collapses into a constant.)

Sharding: data-parallel over batch (16 batches / 8 cores = 2 per core);
weights + bias replicated per core.  All compute in bf16 (measured end-to-end
rel err 2.9e-3 vs the f32 reference); the bias ships as fp8e4m3 (3.3e-3),
halving the dominant HBM stream.

Per-core pipeline (all matmul cost = moving-dim size; stationary loads free):
  A) V[s,b,e] = hidden^T-tiles (stationary) x wvT (moving 512)     34.1us PE
  B) exp(biasT) on ACT (fp8 in, bf16 out);  per (head, i-tile):
     O[i,(b,d)] = PT[j,i]-tiles (stationary) x V[j,(b,d)] (moving 128),
     sumexp via ones-column matmuls (moving 1) into a small PSUM;
     normalize with per-partition tensor_scalar (inv from nc.vector.reciprocal)
                                                                   21.6us PE
  C) O -> OT via DMA-engine xbar transpose (5 instrs, not PE);
     out^T[e_out,(b,i)] = woT-tiles (stationary) x OT (moving)     30.8us PE
Host does the pure layout transforms (transposes, dtype casts) on numpy.
"""

import numpy as np

B, S, E = 16, 577, 1024
H, D = 16, 64
NCORES = 8
BPC = B // NCORES  # batches per core
NKT = E // 128     # contraction tiles
NET = E // 128     # e_out tiles
STILES = [(0, 128), (128, 256), (256, 384), (384, 512), (512, 577)]
NST = len(STILES)


def _build_program(debug=False):
    import concourse.bass as bass
    import concourse.bacc as bacc
    import concourse.mybir as mybir
    import concourse.tile as tile

    dt = mybir.dt
    f32 = dt.float32
    bf16 = dt.bfloat16
    f8 = dt.float8e4
    Exp = mybir.ActivationFunctionType.Exp
    PSUM = bass.MemorySpace.PSUM

    nc = bacc.Bacc("TRN2", target_bir_lowering=False, debug=False, num_devices=NCORES)

    hTd = nc.declare_dram_parameter("hiddenT", [BPC, E, S], bf16, isOutput=False)
    wvd = nc.declare_dram_parameter("wvT", [E, E], bf16, isOutput=False)   # [e_in, e_out]
    wod = nc.declare_dram_parameter("woT", [E, E], bf16, isOutput=False)   # [e_in, e_out]
    vbd = nc.declare_dram_parameter("v_b", [E], bf16, isOutput=False)
    obd = nc.declare_dram_parameter("out_b", [E], f32, isOutput=False)
    btd = nc.declare_dram_parameter("biasT8", [H, S, S], f8, isOutput=False)  # [h, j, i]
    outd = nc.declare_dram_parameter("outT", [BPC, E, S], f32, isOutput=True)

    with tile.TileContext(nc) as tc:
        with (
            tc.tile_pool(name="const", bufs=1) as cpool,
            tc.tile_pool(name="wop", bufs=1) as wopool,
            tc.tile_pool(name="ptp", bufs=8) as ptpool,
            tc.tile_pool(name="stg", bufs=4) as stgpool,
            tc.tile_pool(name="vp", bufs=1) as vpool,
            tc.tile_pool(name="ps", bufs=1, space=PSUM) as pspool,
        ):
            ones = cpool.tile([128, 1], bf16, tag="ones", name="ones")
            nc.vector.memset(ones[:], 1.0)
            vb_sb = cpool.tile([128, NKT], bf16, tag="vb", name="vb")
            ob_sb = cpool.tile([128, NET], f32, tag="ob", name="ob")
            c_sb = cpool.tile([128, NET], f32, tag="c", name="c")
            inv_sb = cpool.tile([128, NST, H], f32, tag="inv", name="inv")
            v_sb = vpool.tile([128, NST, BPC, E], bf16, tag="v", name="v")
            wo_sb = wopool.tile([128, NKT, E], bf16, tag="wo", name="wo")

            pt_t = []
            stage_of = {}

            def stage_dma(hp):
                stg = stgpool.tile([128, BPC, NST, S], f8, tag="stg", name="stg")
                for hh in range(2):
                    nc.sync.dma_start(
                        stg[:, hh, 0:4, :],
                        btd[2 * hp + hh, 0:512, :].rearrange(
                            "(jt p) i -> p jt i", p=128
                        ),
                    )
                nc.sync.dma_start(
                    stg[0:65, :, 4, :],
                    btd[2 * hp : 2 * hp + 2, 512:577, :].rearrange("h p i -> p h i"),
                )
                stage_of[hp] = stg

            def exp_emit(hp):
                p = ptpool.tile([128, 2, NST, S], bf16, tag="pt", name="pt")
                stg = stage_of[hp]
                nc.scalar.activation(p[:, :, 0:4, :], stg[:, :, 0:4, :], Exp)
                nc.scalar.activation(p[0:65, :, 4, :], stg[0:65, :, 4, :], Exp)
                pt_t.append(p)

            # ---- phase A: V projection -----------------------------------
            with (
                tc.tile_pool(name="wvp", bufs=1) as wvpool,
                tc.tile_pool(name="htp", bufs=1) as htpool,
            ):
                wv_sb = wvpool.tile([128, NKT, E], bf16, tag="wv", name="wv")
                ht_sb = htpool.tile([128, NKT, BPC, S], bf16, tag="ht", name="ht")

                # DMA queue order = emission order; get PE + ACT going early.
                nc.sync.dma_start(
                    wv_sb[:, :, 0:512],
                    wvd[:, 0:512].rearrange("(kt p) e -> p kt e", p=128),
                )
                stage_dma(0)
                nc.sync.dma_start(
                    wv_sb[:, :, 512:1024],
                    wvd[:, 512:1024].rearrange("(kt p) e -> p kt e", p=128),
                )
                nc.sync.dma_start(
                    ht_sb[:, :, 0, 0:128],
                    hTd[0, :, 0:128].rearrange("(kt p) s -> p kt s", p=128),
                )
                nc.sync.dma_start(
                    ht_sb[:, :, 0, 128:577],
                    hTd[0, :, 128:577].rearrange("(kt p) s -> p kt s", p=128),
                )
                stage_dma(1)
                nc.sync.dma_start(
                    ht_sb[:, :, 1, :],
                    hTd[1].rearrange("(kt p) s -> p kt s", p=128),
                )
                stage_dma(2)
                stage_dma(3)
                nc.sync.dma_start(
                    wo_sb[:, :, 0:512],
                    wod[:, 0:512].rearrange("(kt p) e -> p kt e", p=128),
                )
                nc.sync.dma_start(
                    wo_sb[:, :, 512:1024],
                    wod[:, 512:1024].rearrange("(kt p) e -> p kt e", p=128),
                )
                nc.sync.dma_start(vb_sb[:], vbd.rearrange("(kt p) -> p kt", p=128))
                nc.sync.dma_start(ob_sb[:], obd.rearrange("(et p) -> p et", p=128))
                # remaining bias streams; stage pool rotation (bufs=4) paces
                # these behind the exp consumer automatically.
                for hp in range(4, 8):
                    stage_dma(hp)

                for hp in range(8):
                    exp_emit(hp)

                for b in range(BPC):
                    for st, (s0, s1) in enumerate(STILES):
                        ssz = s1 - s0
                        for ec in range(2):
                            ps = pspool.tile(
                                [128, 512], f32, tag="A", name="vps", bufs=2
                            )
                            for kt in range(NKT):
                                nc.tensor.matmul(
                                    ps[0:ssz, :],
                                    ht_sb[:, kt, b, s0:s1],
                                    wv_sb[:, kt, 512 * ec : 512 * ec + 512],
                                    start=(kt == 0),
                                    stop=(kt == NKT - 1),
                                )
                            nc.vector.tensor_copy(
                                v_sb[0:ssz, st, b, 512 * ec : 512 * ec + 512],
                                ps[0:ssz, :],
                            )

            # ---- phase B: attention --------------------------------------
            with (
                tc.tile_pool(name="op", bufs=1) as opool,
                tc.tile_pool(name="ostg", bufs=2) as ostgpool,
            ):
                o_sb = opool.tile([128, NST, 8, BPC, 128], bf16, tag="o", name="o")
                ot_sb = opool.tile([128, 8, BPC, NST, 128], bf16, tag="ot", name="ot")
                # it=4 transpose input reads partitions 65:128 (never computed;
                # engine partition offsets must be 32-aligned, so clear 64:128
                # and let the real row-64 writes land on top)
                nc.gpsimd.memset(o_sb[64:128, 4, :, :, :], 0.0)

                for hp in range(8):
                    p = pt_t[hp]
                    # sumexp first (tiny moving-1 matmuls), so the reciprocals
                    # are ready when the main chains' normalize runs.
                    pss = pspool.tile([128, NST, 2], f32, tag="S", name="pss", bufs=2)
                    for hh in range(2):
                        for it, (i0, i1) in enumerate(STILES):
                            isz = i1 - i0
                            for jt, (j0, j1) in enumerate(STILES):
                                jsz = j1 - j0
                                nc.tensor.matmul(
                                    pss[0:isz, it, hh : hh + 1],
                                    p[0:jsz, hh, jt, i0:i1],
                                    ones[0:jsz, :],
                                    start=(jt == 0),
                                    stop=(jt == NST - 1),
                                )
                    # batched reciprocals: full i-tiles in one shot + ragged
                    nc.vector.reciprocal(
                        inv_sb[:, 0:4, 2 * hp : 2 * hp + 2], pss[:, 0:4, :]
                    )
                    nc.vector.reciprocal(
                        inv_sb[0:65, 4, 2 * hp : 2 * hp + 2], pss[0:65, 4, :]
                    )
                    for hh in range(2):
                        h = 2 * hp + hh
                        for it, (i0, i1) in enumerate(STILES):
                            isz = i1 - i0
                            ps = pspool.tile(
                                [128, BPC, 64], f32, tag="B", name="aps", bufs=4
                            )
                            for jt, (j0, j1) in enumerate(STILES):
                                jsz = j1 - j0
                                nc.tensor.matmul(
                                    ps[0:isz, :, :],
                                    p[0:jsz, hh, jt, i0:i1],
                                    v_sb[0:jsz, jt, :, 64 * h : 64 * h + 64],
                                    start=(jt == 0),
                                    stop=(jt == NST - 1),
                                )
                            # normalize; late head-pairs offload half to ACT
                            # (exp is finished by then, GPSIMD can't read PSUM)
                            if hp >= 6 and (it + hh) % 2 == 1:
                                nc.scalar.activation(
                                    o_sb[0:isz, it, hp, :, 64 * hh : 64 * hh + 64],
                                    ps[0:isz, :, :],
                                    mybir.ActivationFunctionType.Copy,
                                    scale=inv_sb[0:isz, it, h : h + 1],
                                )
                            else:
                                nc.vector.tensor_scalar_mul(
                                    o_sb[0:isz, it, hp, :, 64 * hh : 64 * hh + 64],
                                    ps[0:isz, :, :],
                                    inv_sb[0:isz, it, h : h + 1],
                                )

                # O -> OT on the DMA xbar: in [128, 2048] -> out chunks of 128
                # partitions;  chunk t = (hp, b) matches OT's k-tile layout.
                for it in range(NST):
                    nc.sync.dma_start_transpose(
                        ot_sb[:, :, :, it, :],
                        o_sb[:, it, :, :, :].rearrange("p a b c -> p (a b c)"),
                    )

                if debug:
                    dbg_ot = nc.declare_dram_parameter(
                        "dbg_ot", [128, 8, BPC, NST, 128], bf16, isOutput=True
                    )
                    nc.sync.dma_start(dbg_ot[:], ot_sb[:])

                # ---- c = v_b @ woT + out_b (tiny moving-1 chains) --------
                psc = pspool.tile([128, BPC, 64], f32, tag="B", name="cps", bufs=4)
                pscf = psc.rearrange("p a b -> p (a b)")
                for et in range(NET):
                    for kt in range(NKT):
                        nc.tensor.matmul(
                            pscf[:, et : et + 1],
                            wo_sb[:, kt, 128 * et : 128 * et + 128],
                            vb_sb[:, kt : kt + 1],
                            start=(kt == 0),
                            stop=(kt == NKT - 1),
                        )
                nc.vector.tensor_add(c_sb[:], pscf[:, 0:NET], ob_sb[:])

                # ---- phase C: output projection (out^T layout) -----------
                for et in range(NET):
                    ostage = ostgpool.tile([128, BPC, S], f32, tag="os", name="os")
                    for b in range(BPC):
                        ps1 = pspool.tile([128, 512], f32, tag="A", name="ops", bufs=2)
                        for kt in range(NKT):
                            nc.tensor.matmul(
                                ps1[:, :],
                                wo_sb[:, kt, 128 * et : 128 * et + 128],
                                ot_sb[:, kt, b, 0:4, :],
                                start=(kt == 0),
                                stop=(kt == NKT - 1),
                            )
                        nc.vector.tensor_scalar_add(
                            ostage[:, b, 0:512], ps1[:, :], c_sb[:, et : et + 1]
                        )
                        ps2 = pspool.tile([128, BPC, 64], f32, tag="B", name="op2", bufs=4)
                        ps2f = ps2.rearrange("p a b -> p (a b)")
                        for kt in range(NKT):
                            nc.tensor.matmul(
                                ps2f[:, 0:65],
                                wo_sb[:, kt, 128 * et : 128 * et + 128],
                                ot_sb[:, kt, b, 4, 0:65],
                                start=(kt == 0),
                                stop=(kt == NKT - 1),
                            )
                        nc.vector.tensor_scalar_add(
                            ostage[:, b, 512:577], ps2f[:, 0:65], c_sb[:, et : et + 1]
                        )
                    nc.sync.dma_start(
                        outd[:, 128 * et : 128 * et + 128, :].rearrange(
                            "b p s -> p b s"
                        ),
                        ostage[:],
                    )

    nc.finalize()
    return nc


_NC_CACHE = None


def _get_program():
    global _NC_CACHE
    if _NC_CACHE is None:
        _NC_CACHE = _build_program()
    return _NC_CACHE


def kernel(
    hidden_states,
    q_w,
    q_b,
    v_w,
    v_b,
    out_w,
    out_b,
    share_key,
    share_bias,
    layer,
    _trace=False,
):
    """Full-input / full-output entry point. q_w/q_b/share_key/layer are
    mathematically irrelevant (softmax shift invariance) and unused."""
    import ml_dtypes
    from concourse.bass_utils import run_bass_kernel_spmd

    bf = ml_dtypes.bfloat16
    f8 = ml_dtypes.float8_e4m3

    # host-side layout transforms (transposes + dtype casts, no math)
    hiddenT = np.ascontiguousarray(
        np.asarray(hidden_states, np.float32).transpose(0, 2, 1)
    ).astype(bf)
    wvT = np.ascontiguousarray(np.asarray(v_w, np.float32).T).astype(bf)
    woT = np.ascontiguousarray(np.asarray(out_w, np.float32).T).astype(bf)
    vb = np.asarray(v_b, np.float32).astype(bf)
    ob = np.ascontiguousarray(np.asarray(out_b, np.float32))
    biasT8 = np.ascontiguousarray(
        np.asarray(share_bias, np.float32).transpose(0, 2, 1)
    ).astype(f8)

    nc = _get_program()
    in_maps = []
    for c in range(NCORES):
        in_maps.append(
            {
                "hiddenT": hiddenT[c * BPC : (c + 1) * BPC],
                "wvT": wvT,
                "woT": woT,
                "v_b": vb,
                "out_b": ob,
                "biasT8": biasT8,
            }
        )
    res = run_bass_kernel_spmd(nc, in_maps, list(range(NCORES)), trace=_trace)
    outT = np.concatenate([res.results[c]["outT"] for c in range(NCORES)], axis=0)
    if _trace:
        kernel.last_results = res
    return np.ascontiguousarray(outT.transpose(0, 2, 1))


# revision 11
# speedup vs baseline: 1.3035x; 1.1408x over previous
"""CLIP attention (ShareKey branch) Trainium2 Bass kernel, 8-core SPMD.

Math: in the reference, attn = softmax(scores[..., None] + share_bias, axis=-1)
where scores is constant along the softmax axis -> softmax shift-invariance
makes the q-projection / share_key / scores irrelevant. The output is exactly

    P[h]   = softmax(share_bias[h], axis=-1)            (batch independent)
    V[b]   = hidden[b] @ v_w.T                          (v_b folded into c)
    O[b,h] = P[h] @ V[b,h]                              (per-head slice of V)
    out[b] = concat_h(O[b,h]) @ out_w.T + c,  c = v_b @ out_w.T + out_b
    (P rows sum to 1, so the v_b contribution collapses into a constant.)

Sharding: data-parallel over batch (16 batches / 8 cores = 2 per core);
weights + bias replicated per core.  All compute in bf16 (measured end-to-end
rel err 2.9e-3 vs the f32 reference); the bias ships as fp8e4m3 (3.3e-3),
halving the dominant HBM stream.

Per-core pipeline (all matmul cost = moving-dim size; stationary loads free):
  A) V[s,b,e] = hidden^T-tiles (stationary) x wvT (moving 512)     34.1us PE
  B) exp(biasT) on ACT (fp8 in, bf16 out);  per (head, i-tile):
     O[i,(b,d)] = PT[j,i]-tiles (stationary) x V[j,(b,d)] (moving 128),
     sumexp via ones-column matmuls (moving 1) into a small PSUM;
     normalize with per-partition tensor_scalar (inv from nc.vector.reciprocal)
                                                                   21.6us PE
  C) O -> OT via DMA-engine xbar transpose (5 instrs, not PE);
     out^T[e_out,(b,i)] = woT-tiles (stationary) x OT (moving)     30.8us PE
Host does the pure layout transforms (transposes, dtype casts) on numpy.
"""

import numpy as np

B, S, E = 16, 577, 1024
H, D = 16, 64
NCORES = 8
BPC = B // NCORES  # batches per core
NKT = E // 128     # contraction tiles
NET = E // 128     # e_out tiles
STILES = [(0, 128), (128, 256), (256, 384), (384, 512), (512, 577)]
NST = len(STILES)


def _build_program(debug=False):
    import concourse.bass as bass
    import concourse.bacc as bacc
    import concourse.mybir as mybir
    import concourse.tile as tile

    dt = mybir.dt
    f32 = dt.float32
    bf16 = dt.bfloat16
    f8 = dt.float8e4
    Exp = mybir.ActivationFunctionType.Exp
    PSUM = bass.MemorySpace.PSUM

    nc = bacc.Bacc("TRN2", target_bir_lowering=False, debug=False, num_devices=NCORES)

    # hidden^T in 256-wide s-chunks (host-padded) so every DMA row is a
    # contiguous >=512B run (full DMA rate); chunk 2 holds the ragged 65.
    hTd = nc.declare_dram_parameter("hiddenT", [BPC, 3, E, 256], bf16, isOutput=False)
    wvd = nc.declare_dram_parameter("wvT", [E, E], bf16, isOutput=False)   # [e_in, e_out]
    wod = nc.declare_dram_parameter("woT", [E, E], bf16, isOutput=False)   # [e_in, e_out]
    vbd = nc.declare_dram_parameter("v_b", [E], bf16, isOutput=False)
    obd = nc.declare_dram_parameter("out_b", [E], f32, isOutput=False)
    btd = nc.declare_dram_parameter("biasT8", [H, S, S], f8, isOutput=False)  # [h, j, i]
    outd = nc.declare_dram_parameter("outT", [BPC, E, S], f32, isOutput=True)

    with tile.TileContext(nc) as tc:
        with (
            tc.tile_pool(name="const", bufs=1) as cpool,
            tc.tile_pool(name="wop", bufs=1) as wopool,
            tc.tile_pool(name="ptp", bufs=8) as ptpool,
            tc.tile_pool(name="stg", bufs=4) as stgpool,
            tc.tile_pool(name="vp", bufs=1) as vpool,
            tc.tile_pool(name="ps", bufs=1, space=PSUM) as pspool,
        ):
            ones = cpool.tile([128, 1], bf16, tag="ones", name="ones")
            nc.vector.memset(ones[:], 1.0)
            vb_sb = cpool.tile([128, NKT], bf16, tag="vb", name="vb")
            ob_sb = cpool.tile([128, NET], f32, tag="ob", name="ob")
            c_sb = cpool.tile([128, NET], f32, tag="c", name="c")
            inv_sb = cpool.tile([128, NST, H], f32, tag="inv", name="inv")
            v_sb = vpool.tile([128, NST, BPC, E], bf16, tag="v", name="v")
            wo_sb = wopool.tile([128, NKT, E], bf16, tag="wo", name="wo")

            pt_t = []
            stage_of = {}

            def stage_dma(hp):
                stg = stgpool.tile([128, BPC, NST, S], f8, tag="stg", name="stg")
                for hh in range(2):
                    nc.sync.dma_start(
                        stg[:, hh, 0:4, :],
                        btd[2 * hp + hh, 0:512, :].rearrange(
                            "(jt p) i -> p jt i", p=128
                        ),
                    )
                nc.sync.dma_start(
                    stg[0:65, :, 4, :],
                    btd[2 * hp : 2 * hp + 2, 512:577, :].rearrange("h p i -> p h i"),
                )
                stage_of[hp] = stg

            def exp_emit(hp):
                p = ptpool.tile([128, 2, NST, S], bf16, tag="pt", name="pt")
                stg = stage_of[hp]
                nc.scalar.activation(p[:, :, 0:4, :], stg[:, :, 0:4, :], Exp)
                nc.scalar.activation(p[0:65, :, 4, :], stg[0:65, :, 4, :], Exp)
                pt_t.append(p)

            # ---- phase A: V projection -----------------------------------
            with (
                tc.tile_pool(name="wvp", bufs=1) as wvpool,
                tc.tile_pool(name="htp", bufs=1) as htpool,
            ):
                wv_sb = wvpool.tile([128, NKT, E], bf16, tag="wv", name="wv")
                ht_sb = htpool.tile([128, NKT, BPC, S], bf16, tag="ht", name="ht")

                # DMA queue order = emission order; get PE + ACT going early.
                def wv_dma(q):
                    nc.sync.dma_start(
                        wv_sb[:, :, 256 * q : 256 * q + 256],
                        wvd[:, 256 * q : 256 * q + 256].rearrange(
                            "(kt p) e -> p kt e", p=128
                        ),
                    )

                def ht_dma(b, ch):
                    s0, w = 256 * ch, min(256, S - 256 * ch)
                    nc.sync.dma_start(
                        ht_sb[:, :, b, s0 : s0 + w],
                        hTd[b, ch, :, 0:w].rearrange("(kt p) s -> p kt s", p=128),
                    )

                wv_dma(0)
                ht_dma(0, 0)
                wv_dma(1)
                ht_dma(0, 1)
                stage_dma(0)
                wv_dma(2)
                ht_dma(0, 2)
                wv_dma(3)
                stage_dma(1)
                ht_dma(1, 0)
                ht_dma(1, 1)
                stage_dma(2)
                ht_dma(1, 2)
                stage_dma(3)
                nc.sync.dma_start(
                    wo_sb[:, :, 0:512],
                    wod[:, 0:512].rearrange("(kt p) e -> p kt e", p=128),
                )
                nc.sync.dma_start(
                    wo_sb[:, :, 512:1024],
                    wod[:, 512:1024].rearrange("(kt p) e -> p kt e", p=128),
                )
                nc.sync.dma_start(vb_sb[:], vbd.rearrange("(kt p) -> p kt", p=128))
                nc.sync.dma_start(ob_sb[:], obd.rearrange("(et p) -> p et", p=128))
                # remaining bias streams; stage pool rotation (bufs=4) paces
                # these behind the exp consumer automatically.
                for hp in range(4, 8):
                    stage_dma(hp)

                for hp in range(8):
                    exp_emit(hp)

                for b in range(BPC):
                    for st, (s0, s1) in enumerate(STILES):
                        ssz = s1 - s0
                        for ec in range(2):
                            ps = pspool.tile(
                                [128, 512], f32, tag="A", name="vps", bufs=2
                            )
                            for kt in range(NKT):
                                nc.tensor.matmul(
                                    ps[0:ssz, :],
                                    ht_sb[:, kt, b, s0:s1],
                                    wv_sb[:, kt, 512 * ec : 512 * ec + 512],
                                    start=(kt == 0),
                                    stop=(kt == NKT - 1),
                                )
                            nc.vector.tensor_copy(
                                v_sb[0:ssz, st, b, 512 * ec : 512 * ec + 512],
                                ps[0:ssz, :],
                            )

            # ---- phase B: attention --------------------------------------
            with (
                tc.tile_pool(name="op", bufs=1) as opool,
                tc.tile_pool(name="ostg", bufs=2) as ostgpool,
            ):
                o_sb = opool.tile([128, NST, 8, BPC, 128], bf16, tag="o", name="o")
                ot_sb = opool.tile([128, 8, BPC, NST, 128], bf16, tag="ot", name="ot")
                # it=4 transpose input reads partitions 65:128 (never computed;
                # engine partition offsets must be 32-aligned, so clear 64:128
                # and let the real row-64 writes land on top)
                nc.gpsimd.memset(o_sb[64:128, 4, :, :, :], 0.0)

                for hp in range(8):
                    p = pt_t[hp]
                    # sumexp first (tiny moving-1 matmuls), so the reciprocals
                    # are ready when the main chains' normalize runs.
                    pss = pspool.tile([128, NST, 2], f32, tag="S", name="pss", bufs=2)
                    for hh in range(2):
                        for it, (i0, i1) in enumerate(STILES):
                            isz = i1 - i0
                            for jt, (j0, j1) in enumerate(STILES):
                                jsz = j1 - j0
                                nc.tensor.matmul(
                                    pss[0:isz, it, hh : hh + 1],
                                    p[0:jsz, hh, jt, i0:i1],
                                    ones[0:jsz, :],
                                    start=(jt == 0),
                                    stop=(jt == NST - 1),
                                )
                    # batched reciprocals: full i-tiles in one shot + ragged
                    nc.vector.reciprocal(
                        inv_sb[:, 0:4, 2 * hp : 2 * hp + 2], pss[:, 0:4, :]
                    )
                    nc.vector.reciprocal(
                        inv_sb[0:65, 4, 2 * hp : 2 * hp + 2], pss[0:65, 4, :]
                    )
                    for hh in range(2):
                        h = 2 * hp + hh
                        for it, (i0, i1) in enumerate(STILES):
                            isz = i1 - i0
                            ps = pspool.tile(
                                [128, BPC, 64], f32, tag="B", name="aps", bufs=4
                            )
                            for jt, (j0, j1) in enumerate(STILES):
                                jsz = j1 - j0
                                nc.tensor.matmul(
                                    ps[0:isz, :, :],
                                    p[0:jsz, hh, jt, i0:i1],
                                    v_sb[0:jsz, jt, :, 64 * h : 64 * h + 64],
                                    start=(jt == 0),
                                    stop=(jt == NST - 1),
                                )
                            # normalize; late head-pairs offload half to ACT
                            # (exp is finished by then, GPSIMD can't read PSUM)
                            if hp >= 6 and (it + hh) % 2 == 1:
                                nc.scalar.activation(
                                    o_sb[0:isz, it, hp, :, 64 * hh : 64 * hh + 64],
                                    ps[0:isz, :, :],
                                    mybir.ActivationFunctionType.Copy,
                                    scale=inv_sb[0:isz, it, h : h + 1],
                                )
                            else:
                                nc.vector.tensor_scalar_mul(
                                    o_sb[0:isz, it, hp, :, 64 * hh : 64 * hh + 64],
                                    ps[0:isz, :, :],
                                    inv_sb[0:isz, it, h : h + 1],
                                )

                    # O -> OT on the DMA xbar, incrementally per head-pair so
                    # the out-projection's k-tiles stream in as heads finish.
                    # in [128, 256] -> out chunk t = b matches OT's layout.
                    for it in range(NST):
                        nc.sync.dma_start_transpose(
                            ot_sb[:, hp, :, it, :],
                            o_sb[:, it, hp, :, :].rearrange("p a b -> p (a b)"),
                        )

                if debug:
                    dbg_ot = nc.declare_dram_parameter(
                        "dbg_ot", [128, 8, BPC, NST, 128], bf16, isOutput=True
                    )
                    nc.sync.dma_start(dbg_ot[:], ot_sb[:])

                # ---- c = v_b @ woT + out_b (tiny moving-1 chains) --------
                psc = pspool.tile([128, BPC, 64], f32, tag="B", name="cps", bufs=4)
                pscf = psc.rearrange("p a b -> p (a b)")
                for et in range(NET):
                    for kt in range(NKT):
                        nc.tensor.matmul(
                            pscf[:, et : et + 1],
                            wo_sb[:, kt, 128 * et : 128 * et + 128],
                            vb_sb[:, kt : kt + 1],
                            start=(kt == 0),
                            stop=(kt == NKT - 1),
                        )
                nc.vector.tensor_add(c_sb[:], pscf[:, 0:NET], ob_sb[:])

                # ---- phase C: output projection (out^T layout) -----------
                for et in range(NET):
                    for b in range(BPC):
                        ostage = ostgpool.tile([128, S], f32, tag="os", name="os")
                        ps1 = pspool.tile([128, 512], f32, tag="A", name="ops", bufs=2)
                        for kt in range(NKT):
                            nc.tensor.matmul(
                                ps1[:, :],
                                wo_sb[:, kt, 128 * et : 128 * et + 128],
                                ot_sb[:, kt, b, 0:4, :],
                                start=(kt == 0),
                                stop=(kt == NKT - 1),
                            )
                        nc.vector.tensor_scalar_add(
                            ostage[:, 0:512], ps1[:, :], c_sb[:, et : et + 1]
                        )
                        ps2 = pspool.tile([128, BPC, 64], f32, tag="B", name="op2", bufs=4)
                        ps2f = ps2.rearrange("p a b -> p (a b)")
                        for kt in range(NKT):
                            nc.tensor.matmul(
                                ps2f[:, 0:65],
                                wo_sb[:, kt, 128 * et : 128 * et + 128],
                                ot_sb[:, kt, b, 4, 0:65],
                                start=(kt == 0),
                                stop=(kt == NKT - 1),
                            )
                        nc.vector.tensor_scalar_add(
                            ostage[:, 512:577], ps2f[:, 0:65], c_sb[:, et : et + 1]
                        )
                        nc.sync.dma_start(
                            outd[b, 128 * et : 128 * et + 128, :], ostage[:]
                        )

    nc.finalize()
    return nc


_NC_CACHE = None


def _get_program():
    global _NC_CACHE
    if _NC_CACHE is None:
        _NC_CACHE = _build_program()
    return _NC_CACHE


def kernel(
    hidden_states,
    q_w,
    q_b,
    v_w,
    v_b,
    out_w,
    out_b,
    share_key,
    share_bias,
    layer,
    _trace=False,
):
    """Full-input / full-output entry point. q_w/q_b/share_key/layer are
    mathematically irrelevant (softmax shift invariance) and unused."""
    import ml_dtypes
    from concourse.bass_utils import run_bass_kernel_spmd

    bf = ml_dtypes.bfloat16
    f8 = ml_dtypes.float8_e4m3

    # host-side layout transforms (transposes + dtype casts, no math)
    hT = np.asarray(hidden_states, np.float32).transpose(0, 2, 1)  # [B, E, S]
    hiddenT = np.zeros((B, 3, E, 256), dtype=bf)  # s-chunked, ragged tail padded
    hiddenT[:, 0] = hT[:, :, 0:256].astype(bf)
    hiddenT[:, 1] = hT[:, :, 256:512].astype(bf)
    hiddenT[:, 2, :, 0:65] = hT[:, :, 512:577].astype(bf)
    wvT = np.ascontiguousarray(np.asarray(v_w, np.float32).T).astype(bf)
    woT = np.ascontiguousarray(np.asarray(out_w, np.float32).T).astype(bf)
    vb = np.asarray(v_b, np.float32).astype(bf)
    ob = np.ascontiguousarray(np.asarray(out_b, np.float32))
    biasT8 = np.ascontiguousarray(
        np.asarray(share_bias, np.float32).transpose(0, 2, 1)
    ).astype(f8)

    nc = _get_program()
    in_maps = []
    for c in range(NCORES):
        in_maps.append(
            {
                "hiddenT": hiddenT[c * BPC : (c + 1) * BPC],
                "wvT": wvT,
                "woT": woT,
                "v_b": vb,
                "out_b": ob,
                "biasT8": biasT8,
            }
        )
    res = run_bass_kernel_spmd(nc, in_maps, list(range(NCORES)), trace=_trace)
    outT = np.concatenate([res.results[c]["outT"] for c in range(NCORES)], axis=0)
    if _trace:
        kernel.last_results = res
    return np.ascontiguousarray(outT.transpose(0, 2, 1))
